# revision 34
# baseline (speedup 1.0000x reference)
"""Causal multi-head attention layer (train forward) on 8 Trainium2 NeuronCores.

Sharding: batch (4) x head-group (2 of 8 heads each) -> 8 cores.
Per core (batch b, head group g): project Q^T/K^T [512,S] and V [S,512] from
x_b in bf16 (fp32 PSUM accum), run causal attention head-pair-packed on the PE
array. v2 schedule: the attention inner loop is software-pipelined (scores for
chunk k+1 issue before ctx for chunk k), projection / output-projection
matmuls are interleaved into the exp-paced attention stream as pump units,
normalization runs per head-pair with a fast approximate reciprocal, and the
startup DMA is chunked per feature block so the first matmul starts early.
Host pre-casts weights/x to bf16, sums the two partials per batch, adds bo.
"""
import numpy as np
import ml_dtypes

import concourse.bass as bass
import concourse.tile as tile
from concourse import bacc, mybir
from concourse.bass_utils import run_bass_kernel_spmd

F32 = mybir.dt.float32
BF16 = mybir.dt.bfloat16
AF = mybir.ActivationFunctionType
ALU = mybir.AluOpType

P = 128
D = 1024          # model dim
DC = 512          # per-core head dims (8 heads x 64)
HD = 64
NHC = 8           # heads per core
NPAIR = 4         # head pairs per core
FC = D // P       # 8 feature chunks
OC = DC // P      # 4 outdim chunks (= head pairs)
W = 512           # query window (fp32 PSUM bank)
WT = W // P       # token chunks per window
SCALE = 1.0 / 32.0  # 1/sqrt(D)


def build_nc(S=2048, num_devices=8, with_bv=False):
    NWIN = S // W

    nc = bacc.Bacc("TRN2", target_bir_lowering=False, debug=False,
                   num_devices=num_devices)
    xt = nc.dram_tensor("xt", [P, FC, S], BF16, kind="ExternalInput").ap()
    wq = nc.dram_tensor("wq", [P, FC, DC], BF16, kind="ExternalInput").ap()
    wk = nc.dram_tensor("wk", [P, FC, DC], BF16, kind="ExternalInput").ap()
    wv = nc.dram_tensor("wv", [P, FC, DC], BF16, kind="ExternalInput").ap()
    wo = nc.dram_tensor("wo", [P, OC, D], BF16, kind="ExternalInput").ap()
    bias3 = nc.dram_tensor("bias3", [P, 4 * OC], F32,
                           kind="ExternalInput").ap()
    tri = nc.dram_tensor("tri", [P, P], BF16, kind="ExternalInput").ap()
    out = nc.dram_tensor("out", [S, D], F32, kind="ExternalOutput").ap()

    with tile.TileContext(nc) as tc:
        with tc.tile_pool(name="const", bufs=1) as cst, \
             tc.tile_pool(name="stage", bufs=3) as stg, \
             tc.tile_pool(name="pt", bufs=4) as ptp, \
             tc.tile_pool(name="small", bufs=2) as sml, \
             tc.tile_pool(name="stgp", bufs=2) as stgp, \
             tc.tile_pool(name="psA", bufs=1, space="PSUM") as psA, \
             tc.tile_pool(name="psC", bufs=1, space="PSUM") as psC:

            # --- constants (already bf16/pre-arranged from host) ---
            tri_bf = cst.tile([P, P], BF16, tag="tri")
            b3_sb = cst.tile([P, 4 * OC], F32, tag="bias3")
            bq_sb = b3_sb[:, 0:OC]
            bk_sb = b3_sb[:, OC:2 * OC]
            bv_sb = b3_sb[0:HD, 2 * OC:2 * OC + NHC]
            w_sbs = {}
            for name in ("wq", "wk", "wv"):
                w_sbs[name] = cst.tile([P, FC, DC], BF16, tag=name, name=name)
            wo_sb = cst.tile([P, OC, D], BF16, tag="wo")

            # --- per-window tiles ---
            xT_w, qT_w, kT_w, v_w, ctx_w = [], [], [], [], []
            for j in range(NWIN):
                xT_w.append(cst.tile([P, FC, W], BF16, tag=f"xT{j}",
                                     name=f"xT{j}"))
                qT_w.append(cst.tile([P, OC, W], BF16, tag=f"qT{j}",
                                     name=f"qT{j}"))
                kT_w.append(cst.tile([P, OC, W], BF16, tag=f"kT{j}",
                                     name=f"kT{j}"))
                v_w.append(cst.tile([P, WT, NHC, HD + 1], BF16, tag=f"v{j}",
                                    name=f"v{j}"))
                ctx_w.append(cst.tile([P, NPAIR, W], BF16, tag=f"ctx{j}",
                                      name=f"ctx{j}"))
                nc.vector.memset(v_w[j][:, :, :, HD:HD + 1], 1.0)

            # input DMAs issue on gpsimd (25ns/issue); out-DMAs own sync
            def dmaq():
                return nc.gpsimd

            def drain_copy(out_, in_):
                # PSUM -> SBUF drains on DVE (gpsimd can't read PSUM;
                # Scalar paces attention with exp and must stay clean)
                nc.vector.tensor_copy(out_, in_)

            # ---------- pump-unit queue (PE work interleaved into attention)
            units = []  # list of (label, emit_fn)

            def pump(n=1):
                for _ in range(n):
                    if units:
                        units.pop(0)[1]()

            def pump_until(label):
                while any(u[0] == label for u in units):
                    units.pop(0)[1]()

            # ---------- projection emitters (as queued units) ----------
            def qk_units(j, og, wname, dst, b_sb):
                w_sb = w_sbs[wname]
                ps_ref = []

                def mk_mm(half, fcs):
                    def f():
                        if not ps_ref:
                            ps_ref.append(psA.tile([P, 1024], F32, tag="pj",
                                                   name="pj"))
                        ps = ps_ref[0]
                        oc = og * 2 + half
                        for fc in fcs:
                            nc.tensor.matmul(
                                ps[:, half * W:(half + 1) * W],
                                w_sb[:, fc, oc * P:(oc + 1) * P],
                                xT_w[j][:, fc, :],
                                start=(fc == 0), stop=(fc == FC - 1))
                    return f

                def bias():
                    ps = ps_ref[0]
                    for half in range(2):
                        oc = og * 2 + half
                        nc.vector.tensor_scalar(
                            dst[:, oc, :], ps[:, half * W:(half + 1) * W],
                            b_sb[:, oc:oc + 1], None, ALU.add)

                lbl = (wname, j, og)
                return [(lbl, mk_mm(0, range(0, 4))),
                        (lbl, mk_mm(0, range(4, 8))),
                        (lbl, mk_mm(1, range(0, 4))),
                        (lbl, mk_mm(1, range(4, 8))),
                        (lbl, bias)]

            def v_units(j, tg):
                ps_ref = []

                def mk_mm(half, fcs):
                    def f():
                        if not ps_ref:
                            ps_ref.append(psA.tile([P, 1024], F32, tag="pj",
                                                   name="pj"))
                        ps = ps_ref[0]
                        t = tg * 2 + half
                        for fc in fcs:
                            nc.tensor.matmul(
                                ps[:, half * W:(half + 1) * W],
                                xT_w[j][:, fc, t * P:(t + 1) * P],
                                w_sbs["wv"][:, fc, :],
                                start=(fc == 0), stop=(fc == FC - 1))
                    return f

                def drain():
                    ps = ps_ref[0]
                    dv = ps.rearrange("p (t h n) -> p t h n", t=2, h=NHC)
                    drain_copy(v_w[j][:, tg * 2:tg * 2 + 2, :, 0:HD], dv)

                lbl = ("v", j, tg)
                return [(lbl, mk_mm(0, range(0, 4))),
                        (lbl, mk_mm(0, range(4, 8))),
                        (lbl, mk_mm(1, range(0, 4))),
                        (lbl, mk_mm(1, range(4, 8))),
                        (lbl, drain)]

            def outproj_units(j, t, split_drain=False, ptag="pj"):
                tokc = j * WT + t
                ps_ref = []

                def mk_mm(nb):
                    def f():
                        if not ps_ref:
                            ps_ref.append(psA.tile([P, 1024], F32, tag=ptag,
                                                   name=ptag))
                        ps = ps_ref[0]
                        for pr in range(NPAIR):
                            nc.tensor.matmul(
                                ps[:, nb * W:(nb + 1) * W],
                                ctx_w[j][:, pr, t * P:(t + 1) * P],
                                wo_sb[:, pr, nb * 512:(nb + 1) * 512],
                                start=(pr == 0), stop=(pr == NPAIR - 1))
                    return f

                def drain():
                    ps = ps_ref[0]
                    ost = stg.tile([P, D], F32, tag="ostage")
                    if split_drain:
                        nc.scalar.copy(ost[:, 0:W], ps[:, 0:W])
                        nc.vector.tensor_copy(ost[:, W:D], ps[:, W:D])
                    else:
                        drain_copy(ost[:], ps[:])
                    nc.sync.dma_start(out[tokc * P:(tokc + 1) * P, :], ost[:])

                lbl = ("o", j, t)
                return [(lbl, mk_mm(0)), (lbl, mk_mm(1)), (lbl, drain)]

            def queue_proj(j):
                # window-j projections, pair-0 prerequisites first
                units.extend(qk_units(j, 0, "wq", qT_w[j], bq_sb))
                units.extend(qk_units(j, 0, "wk", kT_w[j], bk_sb))
                units.extend(v_units(j, 0))
                units.extend(v_units(j, 1))
                units.extend(qk_units(j, 1, "wq", qT_w[j], bq_sb))
                units.extend(qk_units(j, 1, "wk", kT_w[j], bk_sb))

            def emit_xt(j):
                q = dmaq()
                q.dma_start(xT_w[j][:], xt[:, :, j * W:(j + 1) * W])

            # ---------- attention for one (window, pair), sw-pipelined ----
            def emit_attention_pair(j, p, pace):
                skc_hi = WT * (j + 1)
                ctx0 = psC.tile([P, W], F32, tag="c0", name="ctx0")
                ctx1 = psC.tile([P, W], F32, tag="c1", name="ctx1")
                stash = [None, None]  # (sp, vs) for chunks k, pending exp/ctx

                def emit_scores(skc):
                    rel = skc * P - j * W
                    vs = max(rel, 0)
                    sp = psA.tile([P, 1024], F32, tag=f"sc{skc % 2}",
                                  name=f"sc{skc % 2}")
                    jk, tk = divmod(skc, WT)
                    nc.tensor.matmul(sp[:, vs:W],
                                     kT_w[jk][0:HD, p, tk * P:(tk + 1) * P],
                                     qT_w[j][0:HD, p, vs:W],
                                     start=True, stop=True)
                    nc.tensor.matmul(sp[:, W + vs:2 * W],
                                     kT_w[jk][HD:P, p, tk * P:(tk + 1) * P],
                                     qT_w[j][HD:P, p, vs:W],
                                     start=True, stop=True)
                    return sp, vs, rel

                def emit_exp(skc, sp, vs, rel):
                    spv = sp.rearrange("p (h n) -> p h n", h=2)
                    pt = ptp.tile([P, 1024], BF16, tag="pt", name="pt")
                    ptv = pt.rearrange("p (h n) -> p h n", h=2)
                    nc.scalar.activation(ptv[:, :, vs:W], spv[:, :, vs:W],
                                         AF.Exp, scale=SCALE)
                    if rel >= 0:
                        # window 0: DVE is busy with norm chains; gpsimd idle
                        tri_eng = nc.gpsimd if j == 0 else nc.vector
                        tri_eng.tensor_tensor(
                            ptv[:, :, rel:rel + P], ptv[:, :, rel:rel + P],
                            tri_bf[:, None, :].to_broadcast([P, 2, P]),
                            ALU.mult)
                    return ptv

                def emit_ctx(skc, ptv, vs):
                    jk, tk = divmod(skc, WT)
                    st0 = (skc == 0)
                    sp0 = (skc == skc_hi - 1)
                    nc.tensor.matmul(ctx0[0:HD + 1, vs:W],
                                     v_w[jk][:, tk, 2 * p, :],
                                     ptv[:, 0, vs:W], start=st0, stop=sp0)
                    nc.tensor.matmul(ctx1[0:HD + 1, vs:W],
                                     v_w[jk][:, tk, 2 * p + 1, :],
                                     ptv[:, 1, vs:W], start=st0, stop=sp0)

                pend = None  # (skc, ptv, vs) waiting for its ctx
                for skc in range(skc_hi):
                    if skc == WT * j:  # first diagonal chunk needs this v
                        pump_until(("v", j, 0))
                        pump_until(("v", j, 1))
                    sp, vs, rel = emit_scores(skc)
                    pace()
                    if pend is not None:
                        emit_ctx(*pend)
                    ptv = emit_exp(skc, sp, vs, rel)
                    pend = (skc, ptv, vs)
                if pend is not None:
                    emit_ctx(*pend)
                return ctx0, ctx1

            # ---------- per-pair drain + normalization ----------
            def emit_norm_pair(j, p, ctx0, ctx1, stgw, chunk_split=False):
                # bc DMAs ride sync early (no out-DMAs yet), gpsimd later,
                # so they never queue behind out-DMAs (priority inversion).
                bq_ = nc.sync if j <= 1 else nc.gpsimd
                # Scalar stays pure-exp (it paces attention); DVE drains psC
                rw = sml.tile([1, 2, W], F32, tag="rw", name="rw")
                rc = sml.tile([1, 2, W], F32, tag="rc", name="rc")
                rcw = sml.tile([1, 2, W], BF16, tag="rcw", name="rcw")
                bc = sml.tile([P, W], BF16, tag="bc", name="bc")
                if chunk_split:
                    # tail: per-head reciprocal chains fire bc halves ASAP;
                    # the no-shift drain rides the now-idle Scalar engine
                    nc.vector.tensor_copy(rw[0:1, 0, :], ctx0[HD:HD + 1, :])
                    nc.vector.reciprocal_approx_fast(rc[0:1, 0, :],
                                                     rw[0:1, 0, :])
                    nc.vector.tensor_copy(rcw[0:1, 0, :], rc[0:1, 0, :])
                    bq_.dma_start(bc[0:HD, :], rcw[0:1, 0, None, :]
                                  .to_broadcast([1, HD, W]))
                    nc.vector.tensor_copy(rw[0:1, 1, :], ctx1[HD:HD + 1, :])
                    nc.vector.reciprocal_approx_fast(rc[0:1, 1, :],
                                                     rw[0:1, 1, :])
                    nc.vector.tensor_copy(rcw[0:1, 1, :], rc[0:1, 1, :])
                    bq_.dma_start(bc[HD:P, :], rcw[0:1, 1, None, :]
                                  .to_broadcast([1, HD, W]))
                    nc.scalar.copy(stgw[0:HD, p, :], ctx0[0:HD, :])
                    nc.vector.tensor_copy(stgw[HD:P, p, :], ctx1[0:HD, :])
                    for t in range(WT):
                        sl = slice(t * P, (t + 1) * P)
                        nc.vector.tensor_tensor(
                            ctx_w[j][:, p, sl], stgw[:, p, sl], bc[:, sl],
                            ALU.mult)
                        if with_bv:
                            nc.vector.tensor_scalar(
                                ctx_w[j][:, p, sl], ctx_w[j][:, p, sl],
                                bv_sb[:, p:p + 1], None, ALU.add)
                else:
                    nc.vector.tensor_copy(stgw[0:HD, p, :], ctx0[0:HD, :])
                    nc.vector.tensor_copy(rw[0:1, 0, :], ctx0[HD:HD + 1, :])
                    nc.vector.tensor_copy(stgw[HD:P, p, :], ctx1[0:HD, :])
                    nc.vector.tensor_copy(rw[0:1, 1, :], ctx1[HD:HD + 1, :])
                    nc.vector.reciprocal_approx_fast(rc[0:1, 0, :],
                                                     rw[0:1, 0, :])
                    nc.vector.reciprocal_approx_fast(rc[0:1, 1, :],
                                                     rw[0:1, 1, :])
                    nc.vector.tensor_copy(rcw[0:1, :, :], rc[0:1, :, :])
                    bq_.dma_start(bc[0:HD, :], rcw[0:1, 0, None, :]
                                  .to_broadcast([1, HD, W]))
                    bq_.dma_start(bc[HD:P, :], rcw[0:1, 1, None, :]
                                  .to_broadcast([1, HD, W]))
                    nc.gpsimd.tensor_tensor(ctx_w[j][:, p, :], stgw[:, p, :],
                                            bc[:], ALU.mult)
                    if with_bv:
                        nc.gpsimd.tensor_scalar(
                            ctx_w[j][:, p, :], ctx_w[j][:, p, :],
                            bv_sb[:, p:p + 1], None, ALU.add)

            # ---------- startup: chunked DMA + window-0 projections ------
            for h in range(2):
                q = nc.sync if h == 0 else nc.gpsimd
                for fc in range(h * 4, h * 4 + 4):
                    q.dma_start(xT_w[0][:, fc, :], xt[:, fc, 0:W])
                    q.dma_start(w_sbs["wq"][:, fc, :], wq[:, fc, :])
            for fc in range(FC):
                q = nc.sync if fc % 2 == 0 else nc.gpsimd
                q.dma_start(w_sbs["wk"][:, fc, :], wk[:, fc, :])
                q.dma_start(w_sbs["wv"][:, fc, :], wv[:, fc, :])
            nc.sync.dma_start(tri_bf[:], tri[:])
            nc.sync.dma_start(b3_sb[:], bias3[:])
            nc.gpsimd.dma_start(wo_sb[:], wo[:])
            emit_xt(1)

            # window-0 projections: pair-0 needs first, rest queued
            for u in (qk_units(0, 0, "wq", qT_w[0], bq_sb)
                      + qk_units(0, 0, "wk", kT_w[0], bk_sb)
                      + v_units(0, 0) + v_units(0, 1)):
                u[1]()
            units.extend(qk_units(0, 1, "wq", qT_w[0], bq_sb))
            units.extend(qk_units(0, 1, "wk", kT_w[0], bk_sb))
            queue_proj(1)

            # ---------- main loop ----------
            for j in range(NWIN):
                stgw = stgp.tile([P, NPAIR, W], BF16, tag="stgw", name="stgw")
                if j + 2 < NWIN:
                    emit_xt(j + 2)
                # spread queued PE units evenly across the window's chunks
                chunks_in_win = WT * (j + 1) * NPAIR
                u0 = len(units)
                popped0 = u0
                prog = [0]

                def pace():
                    prog[0] += 1
                    target = u0 * prog[0] // chunks_in_win
                    while (popped0 - len(units)) < target and units:
                        units.pop(0)[1]()

                for p in range(NPAIR):
                    pump_until(("wq", j, p // 2))
                    pump_until(("wk", j, p // 2))
                    ctx0, ctx1 = emit_attention_pair(j, p, pace)
                    last = (j == NWIN - 1 and p == NPAIR - 1)
                    emit_norm_pair(j, p, ctx0, ctx1, stgw, chunk_split=last)
                    pump(1)
                if j + 2 < NWIN:
                    queue_proj(j + 2)
                # outproj(j) consumed two windows later (j+2) where the
                # attention stream has pump slack; window 3 takes 1 and 2
                if j == 1:
                    for t in range(WT):
                        units.extend(outproj_units(0, t))
                elif j == 2:
                    for t in range(WT):
                        units.extend(outproj_units(1, t))
                    for t in range(WT):
                        units.extend(outproj_units(2, t))
                elif j == NWIN - 1:
                    # tail: drain the queue, then final output projection
                    while units:
                        units.pop(0)[1]()
                    for t in range(WT):
                        for u in outproj_units(j, t, split_drain=True,
                                               ptag=f"sc{t % 2}"):
                            u[1]()

    nc.compile()
    return nc


def make_in_maps(x, Wq, bq, Wk, bk, Wv, bv, Wo):
    BF = ml_dtypes.bfloat16
    # tri[p, f] = 1 where f >= p (keep key p for query f within a diag block)
    tri = np.triu(np.ones((P, P), dtype=np.float32)).astype(BF)
    in_maps = []
    for c in range(8):
        b, g = c // 2, c % 2
        sl = slice(g * DC, (g + 1) * DC)
        def warr(w):
            return np.ascontiguousarray(
                w.reshape(-1, P, w.shape[1]).transpose(1, 0, 2)).astype(BF)
        bias3 = np.zeros((P, 4 * OC), np.float32)
        bias3[:, 0:OC] = bq[sl].reshape(OC, P).T
        bias3[:, OC:2 * OC] = bk[sl].reshape(OC, P).T
        bias3[0:HD, 2 * OC:2 * OC + NHC] = bv[sl].reshape(NHC, HD).T
        xtb = np.ascontiguousarray(
            x[b].T.reshape(FC, P, -1).transpose(1, 0, 2)).astype(BF)
        in_maps.append({
            "xt": xtb,
            "wq": warr(Wq[:, sl]),
            "wk": warr(Wk[:, sl]),
            "wv": warr(Wv[:, sl]),
            "wo": warr(Wo[sl, :]),
            "bias3": np.ascontiguousarray(bias3.astype(np.float32)),
            "tri": tri,
        })
    return in_maps


_NC_CACHE = {}


def kernel(x, Wq, bq, Wk, bk, Wv, bv, Wo, bo):
    x = np.asarray(x, dtype=np.float32)
    args = [np.asarray(a, dtype=np.float32)
            for a in (Wq, bq, Wk, bk, Wv, bv, Wo, bo)]
    Wq, bq, Wk, bk, Wv, bv, Wo, bo = args
    key = ("nc", x.shape[1], bool(np.any(bv)))
    if key not in _NC_CACHE:
        _NC_CACHE[key] = build_nc(S=x.shape[1], num_devices=8,
                                  with_bv=bool(np.any(bv)))
    nc = _NC_CACHE[key]
    in_maps = make_in_maps(x, Wq, bq, Wk, bk, Wv, bv, Wo)
    res = run_bass_kernel_spmd(nc, in_maps, core_ids=list(range(8)))
    B = x.shape[0]
    out = np.empty_like(x)
    for b in range(B):
        out[b] = res.results[2 * b]["out"] + res.results[2 * b + 1]["out"] + bo
    return out


# revision 35
# speedup vs baseline: 1.0082x; 1.0082x over previous
"""Causal multi-head attention layer (train forward) on 8 Trainium2 NeuronCores.

Sharding: batch (4) x head-group (2 of 8 heads each) -> 8 cores.
Per core (batch b, head group g): project Q^T/K^T [512,S] and V [S,512] from
x_b in bf16 (fp32 PSUM accum), run causal attention head-pair-packed on the PE
array. v2 schedule: the attention inner loop is software-pipelined (scores for
chunk k+1 issue before ctx for chunk k), projection / output-projection
matmuls are interleaved into the exp-paced attention stream as pump units,
normalization runs per head-pair with a fast approximate reciprocal, and the
startup DMA is chunked per feature block so the first matmul starts early.
Host pre-casts weights/x to bf16, sums the two partials per batch, adds bo.
"""
import numpy as np
import ml_dtypes

import concourse.bass as bass
import concourse.tile as tile
from concourse import bacc, mybir
from concourse.bass_utils import run_bass_kernel_spmd

F32 = mybir.dt.float32
BF16 = mybir.dt.bfloat16
AF = mybir.ActivationFunctionType
ALU = mybir.AluOpType

P = 128
D = 1024          # model dim
DC = 512          # per-core head dims (8 heads x 64)
HD = 64
NHC = 8           # heads per core
NPAIR = 4         # head pairs per core
FC = D // P       # 8 feature chunks
OC = DC // P      # 4 outdim chunks (= head pairs)
W = 512           # query window (fp32 PSUM bank)
WT = W // P       # token chunks per window
SCALE = 1.0 / 32.0  # 1/sqrt(D)


def build_nc(S=2048, num_devices=8, with_bv=False):
    NWIN = S // W

    nc = bacc.Bacc("TRN2", target_bir_lowering=False, debug=False,
                   num_devices=num_devices)
    xt = nc.dram_tensor("xt", [P, FC, S], BF16, kind="ExternalInput").ap()
    wq = nc.dram_tensor("wq", [P, FC, DC], BF16, kind="ExternalInput").ap()
    wk = nc.dram_tensor("wk", [P, FC, DC], BF16, kind="ExternalInput").ap()
    wv = nc.dram_tensor("wv", [P, FC, DC], BF16, kind="ExternalInput").ap()
    wo = nc.dram_tensor("wo", [P, OC, D], BF16, kind="ExternalInput").ap()
    bias3 = nc.dram_tensor("bias3", [P, 4 * OC], F32,
                           kind="ExternalInput").ap()
    tri = nc.dram_tensor("tri", [P, P], BF16, kind="ExternalInput").ap()
    out = nc.dram_tensor("out", [S, D], F32, kind="ExternalOutput").ap()

    with tile.TileContext(nc) as tc:
        with tc.tile_pool(name="const", bufs=1) as cst, \
             tc.tile_pool(name="stage", bufs=3) as stg, \
             tc.tile_pool(name="pt", bufs=4) as ptp, \
             tc.tile_pool(name="small", bufs=2) as sml, \
             tc.tile_pool(name="stgp", bufs=2) as stgp, \
             tc.tile_pool(name="psA", bufs=1, space="PSUM") as psA, \
             tc.tile_pool(name="psC", bufs=1, space="PSUM") as psC:

            # --- constants (already bf16/pre-arranged from host) ---
            tri_bf = cst.tile([P, P], BF16, tag="tri")
            b3_sb = cst.tile([P, 4 * OC], F32, tag="bias3")
            bq_sb = b3_sb[:, 0:OC]
            bk_sb = b3_sb[:, OC:2 * OC]
            bv_sb = b3_sb[0:HD, 2 * OC:2 * OC + NHC]
            w_sbs = {}
            for name in ("wq", "wk", "wv"):
                w_sbs[name] = cst.tile([P, FC, DC], BF16, tag=name, name=name)
            wo_sb = cst.tile([P, OC, D], BF16, tag="wo")

            # --- per-window tiles ---
            xT_w, qT_w, kT_w, v_w, ctx_w = [], [], [], [], []
            for j in range(NWIN):
                xT_w.append(cst.tile([P, FC, W], BF16, tag=f"xT{j}",
                                     name=f"xT{j}"))
                qT_w.append(cst.tile([P, OC, W], BF16, tag=f"qT{j}",
                                     name=f"qT{j}"))
                kT_w.append(cst.tile([P, OC, W], BF16, tag=f"kT{j}",
                                     name=f"kT{j}"))
                v_w.append(cst.tile([P, WT, NHC, HD + 1], BF16, tag=f"v{j}",
                                    name=f"v{j}"))
                ctx_w.append(cst.tile([P, NPAIR, W], BF16, tag=f"ctx{j}",
                                      name=f"ctx{j}"))
                nc.vector.memset(v_w[j][:, :, :, HD:HD + 1], 1.0)

            # input DMAs issue on gpsimd (25ns/issue); out-DMAs own sync
            def dmaq():
                return nc.gpsimd

            def drain_copy(out_, in_):
                # PSUM -> SBUF drains on DVE (gpsimd can't read PSUM;
                # Scalar paces attention with exp and must stay clean)
                nc.vector.tensor_copy(out_, in_)

            # ---------- pump-unit queue (PE work interleaved into attention)
            units = []  # list of (label, emit_fn)

            def pump(n=1):
                for _ in range(n):
                    if units:
                        units.pop(0)[1]()

            def pump_until(label):
                while any(u[0] == label for u in units):
                    units.pop(0)[1]()

            # ---------- projection emitters (as queued units) ----------
            def qk_units(j, og, wname, dst, b_sb):
                w_sb = w_sbs[wname]
                ps_ref = []

                def mk_mm(half, fcs):
                    def f():
                        if not ps_ref:
                            ps_ref.append(psA.tile([P, 1024], F32, tag="pj",
                                                   name="pj"))
                        ps = ps_ref[0]
                        oc = og * 2 + half
                        for fc in fcs:
                            nc.tensor.matmul(
                                ps[:, half * W:(half + 1) * W],
                                w_sb[:, fc, oc * P:(oc + 1) * P],
                                xT_w[j][:, fc, :],
                                start=(fc == 0), stop=(fc == FC - 1))
                    return f

                def bias():
                    ps = ps_ref[0]
                    for half in range(2):
                        oc = og * 2 + half
                        nc.vector.tensor_scalar(
                            dst[:, oc, :], ps[:, half * W:(half + 1) * W],
                            b_sb[:, oc:oc + 1], None, ALU.add)

                lbl = (wname, j, og)
                return [(lbl, mk_mm(0, range(0, 4))),
                        (lbl, mk_mm(0, range(4, 8))),
                        (lbl, mk_mm(1, range(0, 4))),
                        (lbl, mk_mm(1, range(4, 8))),
                        (lbl, bias)]

            def v_units(j, tg):
                ps_ref = []

                def mk_mm(half, fcs):
                    def f():
                        if not ps_ref:
                            ps_ref.append(psA.tile([P, 1024], F32, tag="pj",
                                                   name="pj"))
                        ps = ps_ref[0]
                        t = tg * 2 + half
                        for fc in fcs:
                            nc.tensor.matmul(
                                ps[:, half * W:(half + 1) * W],
                                xT_w[j][:, fc, t * P:(t + 1) * P],
                                w_sbs["wv"][:, fc, :],
                                start=(fc == 0), stop=(fc == FC - 1))
                    return f

                def drain():
                    ps = ps_ref[0]
                    dv = ps.rearrange("p (t h n) -> p t h n", t=2, h=NHC)
                    drain_copy(v_w[j][:, tg * 2:tg * 2 + 2, :, 0:HD], dv)

                lbl = ("v", j, tg)
                return [(lbl, mk_mm(0, range(0, 4))),
                        (lbl, mk_mm(0, range(4, 8))),
                        (lbl, mk_mm(1, range(0, 4))),
                        (lbl, mk_mm(1, range(4, 8))),
                        (lbl, drain)]

            def outproj_units(j, t, split_drain=False, ptag="pj"):
                tokc = j * WT + t
                ps_ref = []

                def mk_mm(nb):
                    def f():
                        if not ps_ref:
                            ps_ref.append(psA.tile([P, 1024], F32, tag=ptag,
                                                   name=ptag))
                        ps = ps_ref[0]
                        for pr in range(NPAIR):
                            nc.tensor.matmul(
                                ps[:, nb * W:(nb + 1) * W],
                                ctx_w[j][:, pr, t * P:(t + 1) * P],
                                wo_sb[:, pr, nb * 512:(nb + 1) * 512],
                                start=(pr == 0), stop=(pr == NPAIR - 1))
                    return f

                def drain():
                    ps = ps_ref[0]
                    ost = stg.tile([P, D], F32, tag="ostage")
                    if split_drain:
                        nc.scalar.copy(ost[:, 0:W], ps[:, 0:W])
                        nc.vector.tensor_copy(ost[:, W:D], ps[:, W:D])
                    else:
                        drain_copy(ost[:], ps[:])
                    nc.sync.dma_start(out[tokc * P:(tokc + 1) * P, :], ost[:])

                lbl = ("o", j, t)
                return [(lbl, mk_mm(0)), (lbl, mk_mm(1)), (lbl, drain)]

            def queue_proj(j):
                # window-j projections, pair-0 prerequisites first
                units.extend(qk_units(j, 0, "wq", qT_w[j], bq_sb))
                units.extend(qk_units(j, 0, "wk", kT_w[j], bk_sb))
                units.extend(v_units(j, 0))
                units.extend(v_units(j, 1))
                units.extend(qk_units(j, 1, "wq", qT_w[j], bq_sb))
                units.extend(qk_units(j, 1, "wk", kT_w[j], bk_sb))

            def emit_xt(j):
                q = dmaq()
                q.dma_start(xT_w[j][:], xt[:, :, j * W:(j + 1) * W])

            # ---------- attention for one (window, pair), sw-pipelined ----
            def emit_attention_pair(j, p, pace):
                skc_hi = WT * (j + 1)
                ctx0 = psC.tile([P, W], F32, tag="c0", name="ctx0")
                ctx1 = psC.tile([P, W], F32, tag="c1", name="ctx1")
                stash = [None, None]  # (sp, vs) for chunks k, pending exp/ctx

                def emit_scores(skc):
                    rel = skc * P - j * W
                    vs = max(rel, 0)
                    sp = psA.tile([P, 1024], F32, tag=f"sc{skc % 2}",
                                  name=f"sc{skc % 2}")
                    jk, tk = divmod(skc, WT)
                    nc.tensor.matmul(sp[:, vs:W],
                                     kT_w[jk][0:HD, p, tk * P:(tk + 1) * P],
                                     qT_w[j][0:HD, p, vs:W],
                                     start=True, stop=True)
                    nc.tensor.matmul(sp[:, W + vs:2 * W],
                                     kT_w[jk][HD:P, p, tk * P:(tk + 1) * P],
                                     qT_w[j][HD:P, p, vs:W],
                                     start=True, stop=True)
                    return sp, vs, rel

                def emit_exp(skc, sp, vs, rel):
                    spv = sp.rearrange("p (h n) -> p h n", h=2)
                    pt = ptp.tile([P, 1024], BF16, tag="pt", name="pt")
                    ptv = pt.rearrange("p (h n) -> p h n", h=2)
                    nc.scalar.activation(ptv[:, :, vs:W], spv[:, :, vs:W],
                                         AF.Exp, scale=SCALE)
                    if rel >= 0:
                        # window 0: DVE is busy with norm chains; gpsimd idle
                        tri_eng = nc.gpsimd if j == 0 else nc.vector
                        tri_eng.tensor_tensor(
                            ptv[:, :, rel:rel + P], ptv[:, :, rel:rel + P],
                            tri_bf[:, None, :].to_broadcast([P, 2, P]),
                            ALU.mult)
                    return ptv

                def emit_ctx(skc, ptv, vs):
                    jk, tk = divmod(skc, WT)
                    st0 = (skc == 0)
                    sp0 = (skc == skc_hi - 1)
                    nc.tensor.matmul(ctx0[0:HD + 1, vs:W],
                                     v_w[jk][:, tk, 2 * p, :],
                                     ptv[:, 0, vs:W], start=st0, stop=sp0)
                    nc.tensor.matmul(ctx1[0:HD + 1, vs:W],
                                     v_w[jk][:, tk, 2 * p + 1, :],
                                     ptv[:, 1, vs:W], start=st0, stop=sp0)

                pend = None  # (skc, ptv, vs) waiting for its ctx
                for skc in range(skc_hi):
                    if skc == WT * j:  # first diagonal chunk needs this v
                        pump_until(("v", j, 0))
                        pump_until(("v", j, 1))
                    sp, vs, rel = emit_scores(skc)
                    pace()
                    if pend is not None:
                        emit_ctx(*pend)
                    ptv = emit_exp(skc, sp, vs, rel)
                    pend = (skc, ptv, vs)
                if pend is not None:
                    emit_ctx(*pend)
                return ctx0, ctx1

            # ---------- per-pair drain + normalization ----------
            def emit_norm_pair(j, p, ctx0, ctx1, stgw, chunk_split=False):
                # bc DMAs ride sync early (no out-DMAs yet), gpsimd later,
                # so they never queue behind out-DMAs (priority inversion).
                bq_ = nc.sync if j <= 1 else nc.gpsimd
                # Scalar stays pure-exp (it paces attention); DVE drains psC
                rw = sml.tile([1, 2, W], F32, tag="rw", name="rw")
                rc = sml.tile([1, 2, W], F32, tag="rc", name="rc")
                rcw = sml.tile([1, 2, W], BF16, tag="rcw", name="rcw")
                bc = sml.tile([P, W], BF16, tag="bc", name="bc")
                if chunk_split:
                    # tail: per-head reciprocal chains fire bc halves ASAP;
                    # the no-shift drain rides the now-idle Scalar engine
                    nc.vector.tensor_copy(rw[0:1, 0, :], ctx0[HD:HD + 1, :])
                    nc.vector.reciprocal_approx_fast(rc[0:1, 0, :],
                                                     rw[0:1, 0, :])
                    nc.vector.tensor_copy(rcw[0:1, 0, :], rc[0:1, 0, :])
                    bq_.dma_start(bc[0:HD, :], rcw[0:1, 0, None, :]
                                  .to_broadcast([1, HD, W]))
                    nc.vector.tensor_copy(rw[0:1, 1, :], ctx1[HD:HD + 1, :])
                    nc.vector.reciprocal_approx_fast(rc[0:1, 1, :],
                                                     rw[0:1, 1, :])
                    nc.vector.tensor_copy(rcw[0:1, 1, :], rc[0:1, 1, :])
                    bq_.dma_start(bc[HD:P, :], rcw[0:1, 1, None, :]
                                  .to_broadcast([1, HD, W]))
                    nc.scalar.copy(stgw[0:HD, p, :], ctx0[0:HD, :])
                    nc.vector.tensor_copy(stgw[HD:P, p, :], ctx1[0:HD, :])
                    for t in range(WT):
                        sl = slice(t * P, (t + 1) * P)
                        nc.vector.tensor_tensor(
                            ctx_w[j][:, p, sl], stgw[:, p, sl], bc[:, sl],
                            ALU.mult)
                        if with_bv:
                            nc.vector.tensor_scalar(
                                ctx_w[j][:, p, sl], ctx_w[j][:, p, sl],
                                bv_sb[:, p:p + 1], None, ALU.add)
                else:
                    nc.vector.tensor_copy(stgw[0:HD, p, :], ctx0[0:HD, :])
                    nc.vector.tensor_copy(rw[0:1, 0, :], ctx0[HD:HD + 1, :])
                    nc.vector.tensor_copy(stgw[HD:P, p, :], ctx1[0:HD, :])
                    nc.vector.tensor_copy(rw[0:1, 1, :], ctx1[HD:HD + 1, :])
                    nc.vector.reciprocal_approx_fast(rc[0:1, 0, :],
                                                     rw[0:1, 0, :])
                    nc.vector.reciprocal_approx_fast(rc[0:1, 1, :],
                                                     rw[0:1, 1, :])
                    nc.vector.tensor_copy(rcw[0:1, :, :], rc[0:1, :, :])
                    bq_.dma_start(bc[0:HD, :], rcw[0:1, 0, None, :]
                                  .to_broadcast([1, HD, W]))
                    bq_.dma_start(bc[HD:P, :], rcw[0:1, 1, None, :]
                                  .to_broadcast([1, HD, W]))
                    nc.gpsimd.tensor_tensor(ctx_w[j][:, p, :], stgw[:, p, :],
                                            bc[:], ALU.mult)
                    if with_bv:
                        nc.gpsimd.tensor_scalar(
                            ctx_w[j][:, p, :], ctx_w[j][:, p, :],
                            bv_sb[:, p:p + 1], None, ALU.add)

            # ---------- startup: chunked DMA + window-0 projections ------
            for h in range(2):
                q = nc.sync if h == 0 else nc.gpsimd
                for fc in range(h * 4, h * 4 + 4):
                    q.dma_start(xT_w[0][:, fc, :], xt[:, fc, 0:W])
                    q.dma_start(w_sbs["wq"][:, fc, :], wq[:, fc, :])
            for h in range(2):
                q = nc.sync if h == 0 else nc.gpsimd
                hs = slice(h * 4, h * 4 + 4)
                q.dma_start(w_sbs["wk"][:, hs, :], wk[:, hs, :])
                q.dma_start(w_sbs["wv"][:, hs, :], wv[:, hs, :])
            nc.sync.dma_start(tri_bf[:], tri[:])
            nc.sync.dma_start(b3_sb[:], bias3[:])
            nc.gpsimd.dma_start(wo_sb[:], wo[:])
            emit_xt(1)

            # window-0 projections: pair-0 needs first, rest queued
            for u in (qk_units(0, 0, "wq", qT_w[0], bq_sb)
                      + qk_units(0, 0, "wk", kT_w[0], bk_sb)
                      + v_units(0, 0) + v_units(0, 1)):
                u[1]()
            units.extend(qk_units(0, 1, "wq", qT_w[0], bq_sb))
            units.extend(qk_units(0, 1, "wk", kT_w[0], bk_sb))
            queue_proj(1)

            # ---------- main loop ----------
            for j in range(NWIN):
                stgw = stgp.tile([P, NPAIR, W], BF16, tag="stgw", name="stgw")
                if j + 2 < NWIN:
                    emit_xt(j + 2)
                # spread queued PE units evenly across the window's chunks
                chunks_in_win = WT * (j + 1) * NPAIR
                u0 = len(units)
                popped0 = u0
                prog = [0]

                def pace():
                    prog[0] += 1
                    target = u0 * prog[0] // chunks_in_win
                    while (popped0 - len(units)) < target and units:
                        units.pop(0)[1]()

                for p in range(NPAIR):
                    pump_until(("wq", j, p // 2))
                    pump_until(("wk", j, p // 2))
                    ctx0, ctx1 = emit_attention_pair(j, p, pace)
                    last = (j == NWIN - 1 and p == NPAIR - 1)
                    emit_norm_pair(j, p, ctx0, ctx1, stgw, chunk_split=last)
                    pump(1)
                if j + 2 < NWIN:
                    queue_proj(j + 2)
                # outproj(j) consumed two windows later (j+2) where the
                # attention stream has pump slack; window 3 takes 1 and 2
                if j == 1:
                    for t in range(WT):
                        units.extend(outproj_units(0, t))
                elif j == 2:
                    for t in range(WT):
                        units.extend(outproj_units(1, t))
                    for t in range(WT):
                        units.extend(outproj_units(2, t))
                elif j == NWIN - 1:
                    # tail: drain the queue, then final output projection
                    while units:
                        units.pop(0)[1]()
                    for t in range(WT):
                        for u in outproj_units(j, t, split_drain=True,
                                               ptag=f"sc{t % 2}"):
                            u[1]()

    nc.compile()
    return nc


def make_in_maps(x, Wq, bq, Wk, bk, Wv, bv, Wo):
    BF = ml_dtypes.bfloat16
    # tri[p, f] = 1 where f >= p (keep key p for query f within a diag block)
    tri = np.triu(np.ones((P, P), dtype=np.float32)).astype(BF)
    in_maps = []
    for c in range(8):
        b, g = c // 2, c % 2
        sl = slice(g * DC, (g + 1) * DC)
        def warr(w):
            return np.ascontiguousarray(
                w.reshape(-1, P, w.shape[1]).transpose(1, 0, 2)).astype(BF)
        bias3 = np.zeros((P, 4 * OC), np.float32)
        bias3[:, 0:OC] = bq[sl].reshape(OC, P).T
        bias3[:, OC:2 * OC] = bk[sl].reshape(OC, P).T
        bias3[0:HD, 2 * OC:2 * OC + NHC] = bv[sl].reshape(NHC, HD).T
        xtb = np.ascontiguousarray(
            x[b].T.reshape(FC, P, -1).transpose(1, 0, 2)).astype(BF)
        in_maps.append({
            "xt": xtb,
            "wq": warr(Wq[:, sl]),
            "wk": warr(Wk[:, sl]),
            "wv": warr(Wv[:, sl]),
            "wo": warr(Wo[sl, :]),
            "bias3": np.ascontiguousarray(bias3.astype(np.float32)),
            "tri": tri,
        })
    return in_maps


_NC_CACHE = {}


def kernel(x, Wq, bq, Wk, bk, Wv, bv, Wo, bo):
    x = np.asarray(x, dtype=np.float32)
    args = [np.asarray(a, dtype=np.float32)
            for a in (Wq, bq, Wk, bk, Wv, bv, Wo, bo)]
    Wq, bq, Wk, bk, Wv, bv, Wo, bo = args
    key = ("nc", x.shape[1], bool(np.any(bv)))
    if key not in _NC_CACHE:
        _NC_CACHE[key] = build_nc(S=x.shape[1], num_devices=8,
                                  with_bv=bool(np.any(bv)))
    nc = _NC_CACHE[key]
    in_maps = make_in_maps(x, Wq, bq, Wk, bk, Wv, bv, Wo)
    res = run_bass_kernel_spmd(nc, in_maps, core_ids=list(range(8)))
    B = x.shape[0]
    out = np.empty_like(x)
    for b in range(B):
        out[b] = res.results[2 * b]["out"] + res.results[2 * b + 1]["out"] + bo
    return out
